# revision 1
# baseline (speedup 1.0000x reference)
"""Trainium2 Bass kernel for nn_DistanceAwareSelfAttentionHead.

Math (reference):
    s  = sigmoid((edge_attr - ib) * im)                    [E]
    rk = Ek1 + s*dEk ; rq = Eq1 + s*dEq ; rv = Ev1 + s*dEv (per edge, rank-1 in s)
    k = x@Wk ; q = x@Wq ; v = x@Wv
    A  = 2 q k^T ; A[src,dst] += q[src].rk + k[dst].rq     (duplicate edges summed)
    P  = softmax(A / sqrt(512))
    M  = P v + segsum(P[src,dst] * rv, src)

Key identities used:
    q[src].rk = a1[src] + s*a2[src]   with a1 = q@Ek1, a2 = q@dEk
    k[dst].rq = b1[dst] + s*b2[dst]   with b1 = k@Eq1, b2 = k@dEq
    summed over duplicates of a cell: bias = m*(a1+b1) + S*(a2+b2),  S = sum_e s_e
    segsum(P*rv, src) row r = u1[r]*Ev1 + u2[r]*dEv, u1 = sum_cells m*P, u2 = sum_cells S*P
    softmax without max-subtraction (logits bounded ~|12|), normalize at the end.

Sharding: rows of A/q/M split across 8 cores (512 rows each); k, v, params
replicated (recomputed per core, no collectives). Edges partitioned by src.

Device layout per core:
    A row-block [128 q-rows, 4096 dst] x 4.  Edge cells are grouped per
    16-row gpsimd core group into a shared "union slot" list (width KG),
    segmented by dst-quadrant (4 x KGQ) so the bias scatter (gpsimd
    local_scatter, per-partition indices) works in [0,1024) chunks.
    Gathers (b1/b2 by dst, P at cells) use gpsimd indirect_copy (per-core
    shared indices).  u1/u2 are tensor_tensor + tensor_reduce; Z comes free
    from the Exp activation's accum_out; division by Z at the very end.
    All fp32 matmul operands are float32r end-to-end (full PE rate); P and v
    are bf16 for the P@v matmul; the P transpose rides the PE in bf16.
"""

import sys

if "/opt/trn_rl_repo" not in sys.path:
    sys.path.insert(0, "/opt/trn_rl_repo")

from contextlib import ExitStack

import numpy as np

import concourse.bacc as bacc
import concourse.mybir as mybir
import concourse.tile as tile
from concourse.bass_utils import run_bass_kernel_spmd

try:
    import ml_dtypes

    BF16_NP = ml_dtypes.bfloat16
except Exception:  # pragma: no cover
    BF16_NP = None

# ---- problem constants (hardcoded per the harness contract) ----
N = 4096
FEAT = 512
HID = 256
NCORES = 8
RPC = N // NCORES  # 512 rows per core
RB = RPC // 128  # 4 row blocks
KGQ = 208  # union slots per (16-row group, dst-quadrant)
KG = 4 * KGQ  # 832 union slots per 16-row group
DW = KG // 16  # 52, wrapped width of gather-index arrays
XW = 32  # max duplicate cells per partition
XSEG = 6  # max extra edges per cell (multiplicity <= 7)
PAD_ATTR = -30.0  # sigmoid(PAD) ~ 1e-13, inside HW ACT table range
SCALE = 1.0 / np.sqrt(np.float32(FEAT))

F32 = mybir.dt.float32
F32R = mybir.dt.float32r
BF16 = mybir.dt.bfloat16
I16 = mybir.dt.int16
U16 = mybir.dt.uint16

# compute-precision knobs
P_BF16 = True  # expA / PT / v / b-tables in bf16 (faster); False = fp32
_CACHE = {}


def _build_nc(p_bf16):
    PD = BF16 if p_bf16 else F32
    nc = bacc.Bacc(
        "TRN2",
        target_bir_lowering=False,
        debug=False,
        enable_asserts=False,
        num_devices=NCORES,
    )
    d = {}

    def din(name, shape, dtype=F32):
        d[name] = nc.dram_tensor(name, shape, dtype, kind="ExternalInput").ap()

    din("xt", [4, 128, N], F32R)  # x^T, feat-major chunks
    din("xtloc", [4, 128, RPC], F32R)  # local columns of x^T
    din("wk", [4, 128, HID], F32R)
    din("wq2", [4, 128, HID], F32R)  # 2*Wq
    din("wv", [4, 128, FEAT], F32R)
    din("ekbh", [2, 128, 2], F32R)  # [Ek1 | dEk] / 2  (chunked over HID)
    din("eqb", [2, 128, 2], F32R)  # [Eq1 | dEq]
    din("ev1bc", [128, FEAT])  # Ev1 broadcast
    din("devbc", [128, FEAT])  # dEv broadcast
    din("ident", [128, 128], PD)
    din("ones1", [1, 128], BF16)
    din("attr0", [RB, 128, KG])  # (attr-ib)*im of first edge, pad PAD_ATTR
    din("mult", [RB, 128, KG], BF16)  # cell multiplicity, 0 on non-owned/pad
    din("idxu", [RB, 128, KG], I16)  # dst%1024 scatter idx, -1 pad
    din("dstp", [RB, 128, DW], U16)  # dst gather idx, wrapped per 16-group
    din("attrx", [RB, 128, XSEG * XW])  # extra-edge attrs (dups), pad PAD_ATTR
    din("sx", [RB, 128, XW], I16)  # union-slot of dup cells, -1 pad
    mout = nc.dram_tensor("mloc", [RPC, FEAT], F32, kind="ExternalOutput").ap()

    AF = mybir.ActivationFunctionType
    OP = mybir.AluOpType

    with tile.TileContext(nc) as tc:
        with ExitStack() as ctx:
            cpool = ctx.enter_context(tc.tile_pool(name="consts", bufs=1))
            wk_t = cpool.tile([128, 4, HID], F32R)
            wq2_t = cpool.tile([128, 4, HID], F32R)
            ekbh_t = cpool.tile([128, 2, 2], F32R)
            eqb_t = cpool.tile([128, 2, 2], F32R)
            ones1_t = cpool.tile([1, 128], BF16)
            ident_t = cpool.tile([128, 128], PD)
            sm_t = cpool.tile([128, 24], F32)  # zA|zB|rz|u1|u2 per row-block
            a12_t = cpool.tile([128, RB, 2], F32)
            ev1bc_t = cpool.tile([128, FEAT], F32)
            devbc_t = cpool.tile([128, FEAT], F32)

            for kc in range(4):
                nc.sync.dma_start(wk_t[:, kc, :], d["wk"][kc])
                nc.sync.dma_start(wq2_t[:, kc, :], d["wq2"][kc])
            for hg in range(2):
                nc.sync.dma_start(ekbh_t[:, hg, :], d["ekbh"][hg])
                nc.sync.dma_start(eqb_t[:, hg, :], d["eqb"][hg])
            nc.sync.dma_start(ones1_t[:], d["ones1"][:])
            nc.sync.dma_start(ident_t[:], d["ident"][:])
            nc.sync.dma_start(ev1bc_t[:], d["ev1bc"][:])
            nc.sync.dma_start(devbc_t[:], d["devbc"][:])

            with tc.tile_pool(name="mid", bufs=1) as pmid:
                qt2_t = pmid.tile([128, 2, RPC], F32R)
                kt_t = pmid.tile([128, 2, N], F32R)
                b1bc_t = pmid.tile([128, N], BF16)
                b2bc_t = pmid.tile([128, N], BF16)
                v_t = pmid.tile([128, 32, FEAT], PD)
                expa_t = pmid.tile([128, RB, N], PD)

                # ---------- phase 1: kT, qT2, v (x^T resident once) --------
                with tc.tile_pool(name="ph1", bufs=1) as p1, tc.tile_pool(
                    name="ps1", bufs=1, space="PSUM"
                ) as ps1:
                    xt_t = p1.tile([128, 4, N], F32R)
                    xtloc_t = p1.tile([128, 4, RPC], F32R)
                    for kc in range(4):
                        nc.sync.dma_start(xtloc_t[:, kc, :], d["xtloc"][kc])
                    # xt arrives in 4 column blocks of 1024 so PE starts early
                    for j in range(4):
                        nc.sync.dma_start(
                            xt_t[:, :, j * 1024 : (j + 1) * 1024],
                            d["xt"][:, :, j * 1024 : (j + 1) * 1024].rearrange(
                                "c p f -> p c f"
                            ),
                        )
                        for hg in range(2):
                            for n in range(2 * j, 2 * j + 2):
                                kps = ps1.tile([128, 512], F32, tag="kps", bufs=2)
                                for kc in range(4):
                                    nc.tensor.matmul(
                                        kps[:],
                                        wk_t[:, kc, hg * 128 : (hg + 1) * 128],
                                        xt_t[:, kc, n * 512 : (n + 1) * 512],
                                        start=(kc == 0),
                                        stop=(kc == 3),
                                    )
                                nc.scalar.copy(
                                    kt_t[:, hg, n * 512 : (n + 1) * 512], kps[:]
                                )
                    for hg in range(2):
                        qps = ps1.tile([128, 512], F32, tag="qps", bufs=2)
                        for kc in range(4):
                            nc.tensor.matmul(
                                qps[:],
                                wq2_t[:, kc, hg * 128 : (hg + 1) * 128],
                                xtloc_t[:, kc, :],
                                start=(kc == 0),
                                stop=(kc == 3),
                            )
                        nc.vector.tensor_copy(qt2_t[:, hg, :], qps[:])
                    for half in range(2):
                        wv_t = p1.tile([128, 4, FEAT // 2], F32R, tag="wvh")
                        for kc in range(4):
                            nc.sync.dma_start(
                                wv_t[:, kc, :],
                                d["wv"][kc][:, half * 256 : (half + 1) * 256],
                            )
                        for mg in range(32):
                            vps = ps1.tile([128, FEAT // 2], F32, tag="vps", bufs=4)
                            for kc in range(4):
                                nc.tensor.matmul(
                                    vps[:],
                                    xt_t[:, kc, mg * 128 : (mg + 1) * 128],
                                    wv_t[:, kc, :],
                                    start=(kc == 0),
                                    stop=(kc == 3),
                                )
                            nc.scalar.copy(
                                v_t[:, mg, half * 256 : (half + 1) * 256], vps[:]
                            )

                # ---------- phase 2a: b tables (bcast) + a12 ----------------
                with tc.tile_pool(name="ph2a", bufs=1) as p2, tc.tile_pool(
                    name="ps2a", bufs=1, space="PSUM"
                ) as ps2:
                    for row, dst_t in ((0, b1bc_t), (1, b2bc_t)):
                        for n in range(8):
                            bps = ps2.tile([1, 512], F32, tag="bps", bufs=2)
                            for hg in range(2):
                                nc.tensor.matmul(
                                    bps[:],
                                    eqb_t[:, hg, row : row + 1],
                                    kt_t[:, hg, n * 512 : (n + 1) * 512],
                                    start=(hg == 0),
                                    stop=(hg == 1),
                                )
                            brow = p2.tile([1, 512], BF16, tag="brow", bufs=2)
                            nc.vector.tensor_copy(brow[:], bps[:])
                            cps = ps2.tile([128, 512], F32, tag="cps", bufs=2)
                            nc.tensor.matmul(
                                cps[:], ones1_t[:], brow[:], start=True, stop=True
                            )
                            nc.scalar.copy(
                                dst_t[:, n * 512 : (n + 1) * 512], cps[:]
                            )
                    for mg in range(RB):
                        aps = ps2.tile([128, 2], F32, tag="aps", bufs=1)
                        for hg in range(2):
                            nc.tensor.matmul(
                                aps[:],
                                qt2_t[:, hg, mg * 128 : (mg + 1) * 128],
                                ekbh_t[:, hg, :],
                                start=(hg == 0),
                                stop=(hg == 1),
                            )
                        nc.vector.tensor_copy(a12_t[:, mg, :], aps[:])

                # ---------- phase 2b: per row-block edge + A + exp + M ------
                with tc.tile_pool(name="edge", bufs=2) as ep, tc.tile_pool(
                    name="ps2b", bufs=1, space="PSUM"
                ) as psb:
                    for rb in range(RB):
                        at0 = ep.tile([128, KG], F32, tag="at0")
                        mu = ep.tile([128, KG], BF16, tag="mu")
                        ixu = ep.tile([128, KG], I16, tag="ixu")
                        dp = ep.tile([128, DW], U16, tag="dp")
                        ax = ep.tile([128, XSEG * XW], F32, tag="ax")
                        sxi = ep.tile([128, XW], I16, tag="sxi")
                        nc.sync.dma_start(at0[:], d["attr0"][rb])
                        nc.sync.dma_start(mu[:], d["mult"][rb])
                        nc.sync.dma_start(ixu[:], d["idxu"][rb])
                        nc.sync.dma_start(dp[:], d["dstp"][rb])
                        nc.sync.dma_start(ax[:], d["attrx"][rb])
                        nc.sync.dma_start(sxi[:], d["sx"][rb])

                        # S = sigmoid(attr0) + scattered extra sigmoids (bf16)
                        sg = ep.tile([128, KG], F32, tag="sg")
                        nc.scalar.activation(sg[:], at0[:], AF.Sigmoid)
                        nc.scalar.activation(ax[:], ax[:], AF.Sigmoid)
                        sE = ep.tile([128, XW], F32, tag="sE")
                        nc.vector.tensor_tensor(
                            sE[:], ax[:, 0:XW], ax[:, XW : 2 * XW], OP.add
                        )
                        for t in range(2, XSEG):
                            nc.vector.tensor_tensor(
                                sE[:], sE[:], ax[:, t * XW : (t + 1) * XW], OP.add
                            )
                        sE16 = ep.tile([128, XW], BF16, tag="sE16")
                        nc.vector.tensor_copy(sE16[:], sE[:])
                        sxd = ep.tile([128, KG], BF16, tag="sxd")
                        nc.gpsimd.local_scatter(sxd[:], sE16[:], sxi[:], 128, KG, XW)
                        nc.vector.tensor_tensor(sg[:], sg[:], sxd[:], OP.add)

                        # gather b1/b2 at cell dst
                        bg1t = ep.tile([128, KG], BF16, tag="bg1")
                        bg2t = ep.tile([128, KG], BF16, tag="bg2")
                        nc.gpsimd.indirect_copy(bg1t[:], b1bc_t[:], dp[:], True)
                        nc.gpsimd.indirect_copy(bg2t[:], b2bc_t[:], dp[:], True)
                        bg1 = bg1t[:]
                        bg2 = bg2t[:]

                        # bias16 = mult*(a1+b1) + S*(a2+b2)   (bf16 chain)
                        tb1 = ep.tile([128, KG], F32, tag="tb1", bufs=1)
                        tb2 = ep.tile([128, KG], F32, tag="tb2", bufs=1)
                        nc.vector.tensor_scalar(
                            tb1[:], bg1, a12_t[:, rb, 0:1], None, OP.add
                        )
                        nc.vector.tensor_tensor(tb1[:], tb1[:], mu[:], OP.mult)
                        nc.vector.tensor_scalar(
                            tb2[:], bg2, a12_t[:, rb, 1:2], None, OP.add
                        )
                        nc.vector.tensor_tensor(tb2[:], tb2[:], sg[:], OP.mult)
                        nc.vector.tensor_tensor(tb1[:], tb1[:], tb2[:], OP.add)
                        b16 = ep.tile([128, KG], BF16, tag="b16", bufs=1)
                        nc.vector.tensor_copy(b16[:], tb1[:])

                        # per dst-quadrant: scatter bias -> B, A matmul, add
                        ac = ep.tile([128, N], F32, tag="ac", bufs=1)
                        for q in range(4):
                            B_t = ep.tile([128, 1024], BF16, tag="B")
                            nc.gpsimd.local_scatter(
                                B_t[:],
                                b16[:, q * KGQ : (q + 1) * KGQ],
                                ixu[:, q * KGQ : (q + 1) * KGQ],
                                128,
                                1024,
                                KGQ,
                            )
                            for nn in range(2):
                                n = 2 * q + nn
                                aps2 = psb.tile([128, 512], F32, tag="apsA", bufs=3)
                                for hg in range(2):
                                    nc.tensor.matmul(
                                        aps2[:],
                                        qt2_t[:, hg, rb * 128 : (rb + 1) * 128],
                                        kt_t[:, hg, n * 512 : (n + 1) * 512],
                                        start=(hg == 0),
                                        stop=(hg == 1),
                                    )
                                nc.vector.tensor_tensor(
                                    ac[:, n * 512 : (n + 1) * 512],
                                    aps2[:],
                                    B_t[:, nn * 512 : (nn + 1) * 512],
                                    OP.add,
                                )
                        # exp halves with Z accumulation
                        nc.scalar.activation(
                            expa_t[:, rb, 0 : N // 2],
                            ac[:, 0 : N // 2],
                            AF.Exp,
                            scale=float(SCALE),
                            accum_out=sm_t[:, rb : rb + 1],
                        )
                        nc.scalar.activation(
                            expa_t[:, rb, N // 2 : N],
                            ac[:, N // 2 : N],
                            AF.Exp,
                            scale=float(SCALE),
                            accum_out=sm_t[:, 4 + rb : 5 + rb],
                        )
                        nc.vector.tensor_tensor(
                            sm_t[:, 8 + rb : 9 + rb],
                            sm_t[:, rb : rb + 1],
                            sm_t[:, 4 + rb : 5 + rb],
                            OP.add,
                        )
                        nc.vector.reciprocal(
                            sm_t[:, 8 + rb : 9 + rb], sm_t[:, 8 + rb : 9 + rb]
                        )

                        # gather P at cells; u1 = sum m*P ; u2 = sum S*P
                        pg = ep.tile([128, KG], PD, tag="pg")
                        nc.gpsimd.indirect_copy(pg[:], expa_t[:, rb, :], dp[:], True)
                        upr = ep.tile([128, KG], BF16, tag="upr", name="upr")
                        nc.vector.tensor_tensor(upr[:], mu[:], pg[:], OP.mult)
                        nc.vector.tensor_reduce(
                            sm_t[:, 12 + rb : 13 + rb], upr[:],
                            mybir.AxisListType.X, OP.add,
                        )
                        upr2 = ep.tile([128, KG], BF16, tag="upr", name="upr2")
                        nc.vector.tensor_tensor(upr2[:], sg[:], pg[:], OP.mult)
                        nc.vector.tensor_reduce(
                            sm_t[:, 16 + rb : 17 + rb], upr2[:],
                            mybir.AxisListType.X, OP.add,
                        )

                        # transposes + M matmuls for this row block
                        mps = psb.tile([128, FEAT], F32, tag="mps", bufs=2)
                        for cg in range(8):
                            tp = psb.tile([128, 512], PD, tag="tp", bufs=3)
                            for j in range(4):
                                c = 4 * cg + j
                                nc.tensor.transpose(
                                    tp[:, j * 128 : (j + 1) * 128],
                                    expa_t[:, rb, c * 128 : (c + 1) * 128],
                                    ident_t[:],
                                )
                            pt = ep.tile([128, 512], PD, tag="pt", bufs=3)
                            if cg % 2 == 0:
                                nc.vector.tensor_copy(pt[:], tp[:])
                            else:
                                nc.scalar.copy(pt[:], tp[:])
                            for j in range(4):
                                c = 4 * cg + j
                                nc.tensor.matmul(
                                    mps[:],
                                    pt[:, j * 128 : (j + 1) * 128],
                                    v_t[:, c, :],
                                    start=(c == 0),
                                    stop=(c == 31),
                                )
                        # combine: (mps + u1*Ev1 + u2*dEv) * rz -> out
                        t1 = ep.tile([128, FEAT], F32, tag="t1", bufs=1)
                        t2 = ep.tile([128, FEAT], F32, tag="t2", bufs=1)
                        mf = ep.tile([128, FEAT], F32, tag="mf", bufs=2)
                        nc.vector.tensor_scalar(
                            t1[:], ev1bc_t[:], sm_t[:, 12 + rb : 13 + rb],
                            None, OP.mult,
                        )
                        nc.vector.tensor_scalar(
                            t2[:], devbc_t[:], sm_t[:, 16 + rb : 17 + rb],
                            None, OP.mult,
                        )
                        nc.vector.tensor_tensor(t1[:], t1[:], t2[:], OP.add)
                        nc.vector.tensor_tensor(t1[:], t1[:], mps[:], OP.add)
                        nc.vector.tensor_scalar(
                            mf[:], t1[:], sm_t[:, 8 + rb : 9 + rb], None, OP.mult
                        )
                        nc.sync.dma_start(mout[rb * 128 : (rb + 1) * 128, :], mf[:])

    nc.compile()
    return nc


def _cumcount(keys):
    """rank of each element within its equal-key group (keys sorted NOT reqd)."""
    order = np.argsort(keys, kind="stable")
    ks = keys[order]
    n = len(ks)
    if n == 0:
        return np.zeros(0, np.int64)
    starts = np.r_[0, np.nonzero(ks[1:] != ks[:-1])[0] + 1]
    lens = np.diff(np.r_[starts, n])
    r = np.arange(n) - np.repeat(starts, lens)
    out = np.empty(n, np.int64)
    out[order] = r
    return out


def _prep(inputs):
    x = np.asarray(inputs["x"], np.float32)
    ei = np.asarray(inputs["edge_index"]).astype(np.int64)
    ea = np.asarray(inputs["edge_attr"], np.float32).reshape(-1)
    Wk = np.asarray(inputs["Wk"], np.float32)
    Wq = np.asarray(inputs["Wq"], np.float32)
    Wv = np.asarray(inputs["Wv"], np.float32)
    Ek = np.asarray(inputs["Ek"], np.float32)
    Eq = np.asarray(inputs["Eq"], np.float32)
    Ev = np.asarray(inputs["Ev"], np.float32)
    ib = float(np.asarray(inputs["idx_bias"]).reshape(()))
    im = float(np.asarray(inputs["idx_mult"]).reshape(()))

    src, dst = ei[0], ei[1]
    # unique (src, dst) cells, duplicates merged structurally
    key = src * N + dst
    order = np.argsort(key, kind="stable")
    ks, eas = key[order], ea[order]
    uq, first, counts = np.unique(ks, return_index=True, return_counts=True)
    u_src = uq // N
    u_dst = uq % N
    a0 = eas[first]

    c = u_src // RPC
    sl = u_src % RPC
    rb = sl // 128
    p = sl % 128
    g16 = p // 16
    q = u_dst // 1024
    gk = ((c * RB + rb) * 8 + g16) * 4 + q
    rank = _cumcount(gk)
    assert rank.max(initial=0) < KGQ, f"KGQ overflow: {rank.max()}"
    assert counts.max(initial=1) <= XSEG + 1, f"multiplicity {counts.max()}"
    col = q * KGQ + rank

    attr0 = np.full((NCORES, RB, 128, KG), PAD_ATTR, np.float32)
    mult = np.zeros((NCORES, RB, 128, KG), BF16_NP if BF16_NP else np.float32)
    idxu = np.full((NCORES, RB, 128, KG), -1, np.int16)
    dstp = np.zeros((NCORES, RB, 128, DW), np.uint16)
    attrx = np.full((NCORES, RB, 128, XSEG * XW), PAD_ATTR, np.float32)
    sx = np.full((NCORES, RB, 128, XW), -1, np.int16)

    attr0[c, rb, p, col] = (a0 - ib) * im
    mult[c, rb, p, col] = counts
    idxu[c, rb, p, col] = (u_dst - q * 1024).astype(np.int16)
    dstp[c, rb, 16 * g16 + col % 16, col // 16] = u_dst.astype(np.uint16)

    dup = counts > 1
    if dup.any():
        pk = ((c * RB + rb) * 128 + p)[dup]
        xslot = _cumcount(pk)
        assert xslot.max(initial=0) < XW, f"XW overflow: {xslot.max()}"
        cd, rbd, pd_, cold = c[dup], rb[dup], p[dup], col[dup]
        fd, ctd = first[dup], counts[dup]
        sx[cd, rbd, pd_, xslot] = cold.astype(np.int16)
        for t in range(XSEG):
            sel = ctd >= t + 2
            if not sel.any():
                break
            attrx[cd[sel], rbd[sel], pd_[sel], t * XW + xslot[sel]] = (
                eas[fd[sel] + t + 1] - ib
            ) * im

    xT = np.ascontiguousarray(x.T)  # [FEAT, N]
    ident_np = np.eye(128, dtype=np.float32)
    if P_BF16:
        assert BF16_NP is not None, "ml_dtypes needed for bf16 path"
        ident_np = ident_np.astype(BF16_NP)
    shared = {
        "xt": np.ascontiguousarray(xT.reshape(4, 128, N)),
        "wk": np.ascontiguousarray(Wk.reshape(4, 128, HID)),
        "wq2": np.ascontiguousarray((2.0 * Wq).reshape(4, 128, HID)),
        "wv": np.ascontiguousarray(Wv.reshape(4, 128, FEAT)),
        "ekbh": np.ascontiguousarray(
            (0.5 * np.stack([Ek[1], Ek[0] - Ek[1]], axis=1)).reshape(2, 128, 2)
        ),
        "eqb": np.ascontiguousarray(
            np.stack([Eq[1], Eq[0] - Eq[1]], axis=1).reshape(2, 128, 2)
        ),
        "ev1bc": np.ascontiguousarray(np.broadcast_to(Ev[1], (128, FEAT))),
        "devbc": np.ascontiguousarray(np.broadcast_to(Ev[0] - Ev[1], (128, FEAT))),
        "ident": ident_np,
        "ones1": np.ones((1, 128), BF16_NP if BF16_NP else np.float32),
    }
    in_maps = []
    for cc in range(NCORES):
        m = dict(shared)
        m["xtloc"] = np.ascontiguousarray(
            xT[:, cc * RPC : (cc + 1) * RPC].reshape(4, 128, RPC)
        )
        m["attr0"] = attr0[cc]
        m["mult"] = mult[cc]
        m["idxu"] = idxu[cc]
        m["dstp"] = dstp[cc]
        m["attrx"] = attrx[cc]
        m["sx"] = sx[cc]
        in_maps.append(m)
    return in_maps


def get_nc():
    key = (P_BF16,)
    if key not in _CACHE:
        _CACHE[key] = _build_nc(P_BF16)
    return _CACHE[key]


def kernel(**inputs) -> np.ndarray:
    nc = get_nc()
    in_maps = _prep(inputs)
    res = run_bass_kernel_spmd(nc, in_maps, list(range(NCORES)))
    return np.concatenate(
        [res.results[cc]["mloc"] for cc in range(NCORES)], axis=0
    ).astype(np.float32)

